# revision 1
# baseline (speedup 1.0000x reference)
"""Causal self-attention kernel for Trainium2, 8-way sharded.

Problem: B=2, T=2048, C=1024, NH=16, hd=64. fp32 in/out.

Sharding: core = (batch b, head-group g of 4 heads). Each core computes its
4 heads' attention for its batch plus the partial output projection
y_local @ Wo[g*256:(g+1)*256, :]; the host sums the 4 partials per batch
(biases bv/bo are folded in exactly via a host-side correction row).

Device design notes:
  - Projection / score matmuls in float32r (full PE rate, ~1e-4 err);
    P^T and V in bf16 (softmax weights tolerate it; ~2e-3 total err).
  - Everything stays transposed (qT/kT/S^T/P^T/y^T): no on-chip transposes.
    Scores: S^T[tk,tq] = kT.T @ qT as K=64 row-tiled pairs (two heads run
    concurrently in different PE row groups).
  - Softmax without max-subtraction (scores are O(1), exp-safe in fp32):
    P^T = exp(S^T/8) on ScalarE, straight PSUM -> SBUF bf16.
  - Causal masking of diagonal-band tiles via GPSIMD affine_select.
  - PV: y^T_aug[72,tq] += V_aug[tk,72].T @ P^T with 8 appended ones columns
    in V_aug -> softmax denominators appear in PSUM rows 64-71; a K=1
    selector matmul moves each window's denominator onto its own row of a
    PSUM "denominator board" (rows 0-7), enabling one batched 8-lane
    reciprocal per pair; K=8 selector matmuls broadcast the reciprocals.
  - The exp stream is the second-largest engine load (~90us on ScalarE vs
    ~110us of matmuls), so the program is emitted as ONE fused loop that
    paces S^T slots at exp speed and fills the PE gaps with lagged PV
    matmuls and projection bursts (v / qk of the second head-pair) at
    instruction granularity.  PE order never depends on a later PE op.
"""
import contextlib

import ml_dtypes
import numpy as np

import concourse.bass as bass
import concourse.tile as tile
from concourse import bacc, mybir
from concourse import bass_utils

bass_utils.upload_artifacts = lambda tmpdir: "local://skipped"

B, T, C = 2, 2048, 1024
NH, HD = 16, 64
NHL = 4            # heads per core
CLOC = NHL * HD    # 256 local channels
NCH = C // 128     # 8 contraction chunks
TQW = 512          # tq window
NW = T // TQW      # 4 windows
NTT = T // 128     # 16 t-tiles / tk-chunks
VSTR = HD + 8      # 72: v cols per head + 8 ones cols (denoms at rows 64-71)
LAG = 2            # PV trails S^T by this many chunk-groups
F32R = mybir.dt.float32r
F32 = mybir.dt.float32
BF16 = mybir.dt.bfloat16

_cache = {}


def _build():
    nc = bacc.Bacc("TRN2", target_bir_lowering=False, debug=False, num_devices=8)

    xt_ap = nc.dram_tensor("xt", [128, NCH * T], F32R, kind="ExternalInput").ap()
    wq_ap = nc.dram_tensor("wq", [128, 2 * NCH * 128], F32R, kind="ExternalInput").ap()
    wk_ap = nc.dram_tensor("wk", [128, 2 * NCH * 128], F32R, kind="ExternalInput").ap()
    wv_ap = nc.dram_tensor("wv", [128, NCH * CLOC], F32R, kind="ExternalInput").ap()
    wo_ap = nc.dram_tensor("wo", [128, 2 * C], F32R, kind="ExternalInput").ap()
    bq_ap = nc.dram_tensor("bq", [2, 128, 1], F32, kind="ExternalInput").ap()
    bk_ap = nc.dram_tensor("bk", [2, 128, 1], F32, kind="ExternalInput").ap()
    ones_ap = nc.dram_tensor("ones", [128, NTT, NHL, 8], BF16, kind="ExternalInput").ap()
    sels_ap = nc.dram_tensor("sels", [128, 512], F32, kind="ExternalInput").ap()
    selc_ap = nc.dram_tensor("selc", [128, 64], F32R, kind="ExternalInput").ap()
    tri_ap = nc.dram_tensor("tri", [128, 128], BF16, kind="ExternalInput").ap()
    out_ap = nc.dram_tensor("out", [T, C], F32, kind="ExternalOutput").ap()

    with tile.TileContext(nc) as tc, contextlib.ExitStack() as ctx:
        sb = ctx.enter_context(tc.tile_pool(name="sb", bufs=1))
        r_pool = ctx.enter_context(tc.tile_pool(name="rp", bufs=1))
        ost_pool = ctx.enter_context(tc.tile_pool(name="ost", bufs=2))
        pt_pool = ctx.enter_context(tc.tile_pool(name="ptp", bufs=10))
        ps = ctx.enter_context(tc.tile_pool(name="ps", bufs=1, space="PSUM"))

        # ---- persistent SBUF tensors ----
        wqs = sb.tile([128, 2 * NCH * 128], F32R, tag="wqs")
        wks = sb.tile([128, 2 * NCH * 128], F32R, tag="wks")
        wvs = sb.tile([128, NCH * CLOC], F32R, tag="wvs")
        wos = sb.tile([128, 2 * C], F32R, tag="wos")
        xts = sb.tile([128, NCH * T], F32R, tag="xts")
        qts = [sb.tile([128, T], F32R, tag=f"qt{p}", name=f"qt{p}") for p in range(2)]
        kts = [sb.tile([128, T], F32R, tag=f"kt{p}", name=f"kt{p}") for p in range(2)]
        vna = sb.tile([128, NTT * NHL * VSTR], BF16, tag="vna")
        yts = [sb.tile([128, T], F32R, tag=f"yt{p}", name=f"yt{p}") for p in range(2)]
        bqs = [sb.tile([128, 1], F32, tag=f"bq{p}", name=f"bqs{p}") for p in range(2)]
        bks = [sb.tile([128, 1], F32, tag=f"bk{p}", name=f"bks{p}") for p in range(2)]
        sels = sb.tile([128, 512], F32, tag="sels")
        selc = sb.tile([128, 64], F32R, tag="selc")
        dsb = sb.tile([128, TQW], F32, tag="dsb")
        tri = sb.tile([128, 128], BF16, tag="tri")
        rcoll = sb.tile([128, TQW], F32, tag="rcoll")

        # ---- input DMAs: spread over the 3 DMA-capable queues; the first
        # matmuls need wq + xt chunk 0, so those go first on their queues.
        nc.sync.dma_start(wqs[:], wq_ap[:])
        nc.gpsimd.dma_start(xts[:, 0:2 * T], xt_ap[:, 0:2 * T])
        nc.scalar.dma_start(xts[:, 2 * T:4 * T], xt_ap[:, 2 * T:4 * T])
        nc.sync.dma_start(wks[:], wk_ap[:])
        nc.gpsimd.dma_start(xts[:, 4 * T:6 * T], xt_ap[:, 4 * T:6 * T])
        nc.scalar.dma_start(xts[:, 6 * T:8 * T], xt_ap[:, 6 * T:8 * T])
        for p in range(2):
            nc.sync.dma_start(bqs[p][:], bq_ap[p])
            nc.sync.dma_start(bks[p][:], bk_ap[p])
        nc.sync.dma_start(sels[:], sels_ap[:])
        nc.sync.dma_start(selc[:], selc_ap[:])
        nc.sync.dma_start(tri[:], tri_ap[:])
        vna4 = vna[:].rearrange("p (t h v) -> p t h v", t=NTT, h=NHL)
        nc.gpsimd.dma_start(vna4[:, :, :, HD:HD + 8], ones_ap[:])
        nc.scalar.dma_start(wvs[:], wv_ap[:])
        nc.scalar.dma_start(wos[:], wo_ap[:])

        pt_tiles = {}
        D_tiles = {}

        # ---------- emission primitives ----------
        def warm():
            wtile = sb.tile([128, 640], BF16, tag="warm")
            wjunk = sb.tile([128, 8], F32, tag="wjunk")
            nc.vector.memset(wtile[:], 0.0)
            wp = ps.tile([128, TQW], F32, tag="work", bufs=1, name="warm_ps")
            for i in range(40):
                nc.tensor.matmul(wp[:], wtile[:, 0:128], wtile[:, 128:640],
                                 start=True, stop=True)
            nc.vector.tensor_copy(wjunk[:], wp[:, 0:8])

        def qk_window(p, ty, w):
            wsb, dst, bias = ((wqs, qts[p], bqs[p]), (wks, kts[p], bks[p]))[ty]
            tag = "st" if p == 0 else "work"
            acc = ps.tile([128, TQW], F32, tag=tag, bufs=2 if p == 0 else 1,
                          name=f"qk{p}{ty}{w}")
            for c in range(NCH):
                nc.tensor.matmul(
                    acc[:], wsb[:, (p * NCH + c) * 128:(p * NCH + c + 1) * 128],
                    xts[:, c * T + w * TQW: c * T + w * TQW + TQW],
                    start=(c == 0), stop=(c == NCH - 1))
            nc.vector.tensor_scalar_add(dst[:, w * TQW:(w + 1) * TQW],
                                        acc[:], bias[:])

        def v_tile(tt):
            acc = ps.tile([128, CLOC], F32, tag="work", bufs=1, name=f"v{tt}")
            for c in range(NCH):
                nc.tensor.matmul(acc[:], xts[:, c * T + tt * 128: c * T + tt * 128 + 128],
                                 wvs[:, c * CLOC:(c + 1) * CLOC],
                                 start=(c == 0), stop=(c == NCH - 1))
            base = tt * NHL * VSTR
            dst = vna[:, base:base + NHL * VSTR].rearrange("p (h d) -> p h d", h=NHL)
            nc.vector.tensor_copy(dst[:, :, 0:HD],
                                  acc[:].rearrange("p (h d) -> p h d", h=NHL))

        def st_slot(p, w, g, h):
            qt, kt = qts[p], kts[p]
            nchunks = 4 * (w + 1)
            c0 = 2 * g
            st = ps.tile([128, 1024], F32, tag="st", bufs=2,
                         name=f"st{p}{w}{g}{h}")
            for j in range(2):
                c = c0 + j
                nc.tensor.matmul(
                    st[:, j * TQW:(j + 1) * TQW],
                    kt[h * 64:(h + 1) * 64, c * 128:(c + 1) * 128],
                    qt[h * 64:(h + 1) * 64, w * TQW:(w + 1) * TQW],
                    start=True, stop=True)
            pt = pt_pool.tile([128, 1024], BF16, tag="pt", name=f"pt{p}{w}{g}{h}")
            nc.scalar.activation(pt[:], st[:], mybir.ActivationFunctionType.Exp,
                                 scale=0.125)
            if c0 + 1 >= nchunks - 4:
                # causal mask: chunk c covers tq in [0,512) of this window,
                # diag 128-block at cols [128*jp, 128*jp+128), left of it = 0
                for j in range(2):
                    jp = (c0 + j) - 4 * w
                    if jp > 0:
                        nc.gpsimd.memset(pt[:, j * TQW: j * TQW + 128 * jp], 0.0)
                    dslc = pt[:, j * TQW + 128 * jp: j * TQW + 128 * jp + 128]
                    nc.vector.tensor_mul(dslc, dslc, tri[:])
            pt_tiles[(p, w, g, h)] = pt

        def pv_group(p, w, g, h, accs):
            nchunks = 4 * (w + 1)
            c0 = 2 * g
            pt = pt_tiles.pop((p, w, g, h))
            for j in range(2):
                c = c0 + j
                vbase = c * NHL * VSTR + (2 * p + h) * VSTR
                nc.tensor.matmul(
                    accs[h][0:VSTR, :],
                    vna[:, vbase:vbase + VSTR],
                    pt[:, j * TQW:(j + 1) * TQW],
                    start=(c0 == 0 and j == 0),
                    stop=(c0 == nchunks - 2 and j == 1))

        def pv_tail(p, w, h, accs):
            yt = yts[p]
            D = D_tiles[p]
            nc.vector.tensor_copy(yt[h * 64:(h + 1) * 64, w * TQW:(w + 1) * TQW],
                                  accs[h][0:HD, :])
            i = 2 * w + h
            rstage = r_pool.tile([128, TQW], F32R, tag="rstage", bufs=2,
                                 name=f"rst{p}{w}{h}")
            nc.vector.tensor_copy(rstage[HD:HD + 1, :], accs[h][HD:HD + 1, :])
            nc.tensor.matmul(D[0:8, :], selc[HD:HD + 1, 8 * i:8 * i + 8],
                             rstage[HD:HD + 1, :], start=False,
                             stop=(i == 7), skip_group_check=True)

        def norm_item(p):
            D = D_tiles.pop(p)
            nc.vector.tensor_copy(dsb[0:8, :], D[0:8, :])
            nc.vector.reciprocal(rcoll[0:8, :], dsb[0:8, :])
            for w in range(NW):
                R = ps.tile([128, TQW], F32, tag="work", bufs=1, name=f"R{p}{w}")
                nc.tensor.matmul(R[:], sels[0:8, w * 128:(w + 1) * 128],
                                 rcoll[0:8, :], start=True, stop=True)
                for h in range(2):
                    yslc = yts[p][h * 64:(h + 1) * 64, w * TQW:(w + 1) * TQW]
                    nc.vector.tensor_mul(yslc, yslc, R[h * 64:(h + 1) * 64, :])

        def out_item():
            for tt in range(NTT):
                po = ps.tile([128, 1024], F32, tag="st", bufs=2, name=f"po{tt}")
                for nh in range(2):
                    for cc in range(2):
                        nc.tensor.matmul(po[:, nh * TQW:(nh + 1) * TQW],
                                         yts[cc][:, tt * 128:(tt + 1) * 128],
                                         wos[:, cc * C + nh * TQW: cc * C + nh * TQW + TQW],
                                         start=(cc == 0), stop=(cc == 1))
                for nh in range(2):
                    ost = ost_pool.tile([128, TQW], F32, tag="ost", name=f"o{tt}{nh}")
                    if (2 * tt + nh) % 2 == 0:
                        nc.vector.tensor_copy(ost[:], po[:, nh * TQW:(nh + 1) * TQW])
                    else:
                        nc.scalar.copy(ost[:], po[:, nh * TQW:(nh + 1) * TQW])
                    deng = (nc.sync, nc.gpsimd, nc.scalar)[(2 * tt + nh) % 3]
                    deng.dma_start(
                        out_ap[tt * 128:(tt + 1) * 128, nh * TQW:(nh + 1) * TQW],
                        ost[:])

        # ---------- fused schedule ----------
        # filler: list of (kind, id, fn) emitting ~1-2us of dense PE work
        filler = []
        for tt in range(NTT):
            filler.append(("v", tt, lambda tt=tt: v_tile(tt)))
        for ty in range(2):
            for w in range(NW):
                filler.append(("qk1", None, lambda ty=ty, w=w: qk_window(1, ty, w)))
        filler_pe = {"v": 0.95, "qk1": 1.84}     # us of PE work per burst

        state = {"deficit": 0.0}  # ACT-emitted minus PE-emitted (us)

        def pull_filler(min_deficit=0.0, need_v=None, need_qk1=False):
            while filler:
                kind, ident, fn = filler[0]
                forced = (need_v is not None and kind == "v" and ident <= need_v) \
                         or (need_qk1 and kind == "qk1")
                if not forced and state["deficit"] < min_deficit:
                    return
                filler.pop(0)
                fn()
                state["deficit"] -= filler_pe[kind]
                if forced:
                    continue

        warm()
        for ty in range(2):
            for w in range(NW):
                qk_window(0, ty, w)

        for p in range(2):
            if p == 1:
                # ensure pair-1 q/k projections are in the PE stream first
                pull_filler(need_v=NTT, need_qk1=True)
            D = ps.tile([128, TQW], F32, tag="D", bufs=1, name=f"D{p}")
            nc.vector.memset(D[0:8, :], 0.0)
            D_tiles[p] = D
            for w in range(NW):
                ngroups = 2 * (w + 1)
                accs = [ps.tile([128, TQW], F32, tag=f"acc{h}", bufs=1,
                                name=f"acc{p}{w}{h}") for h in range(2)]
                for g in range(ngroups + LAG):
                    if g < ngroups:
                        # PV of group g needs v tiles for chunks 2g, 2g+1
                        pull_filler(need_v=2 * g + 1)
                        st_slot(p, w, g, 0)
                        st_slot(p, w, g, 1)
                        state["deficit"] += 2.3 - 0.94
                    if g >= LAG:
                        gg = g - LAG
                        pv_group(p, w, gg, 0, accs)
                        pv_group(p, w, gg, 1, accs)
                        state["deficit"] -= 0.86
                    pull_filler(min_deficit=1.0)
                for h in range(2):
                    pv_tail(p, w, h, accs)
            norm_item(p)
        # any leftover filler (shouldn't be much)
        pull_filler(need_v=NTT, need_qk1=True)
        out_item()

    nc.compile()
    return nc


def _sels():
    s = np.zeros((128, 512), np.float32)
    for w in range(4):
        s[2 * w, w * 128:w * 128 + 64] = 1.0
        s[2 * w + 1, w * 128 + 64:w * 128 + 128] = 1.0
    return s


def _selc():
    s = np.zeros((128, 64), np.float32)
    for i in range(8):
        s[64, 8 * i + i] = 1.0
    return s


def _to_sbuf_chunks(a, nch):
    """[nch*128, F] row-major -> [128, nch*F] SBUF-native layout."""
    n, fdim = a.shape
    assert n == nch * 128
    return np.ascontiguousarray(
        a.reshape(nch, 128, fdim).transpose(1, 0, 2).reshape(128, nch * fdim))


def _prep_core_inputs(b, g, x, Wq, bq, Wk, bk, Wv, bv, Wo, bo):
    f = np.float32
    xt = _to_sbuf_chunks(np.ascontiguousarray(x[b].T, dtype=f), NCH)
    def pack(W, bvec):
        cols = []
        bp = np.empty((2, 128, 1), f)
        for p in range(2):
            h0, h1 = 4 * g + 2 * p, 4 * g + 2 * p + 1
            Wp = np.concatenate([W[:, h0 * HD:(h0 + 1) * HD],
                                 W[:, h1 * HD:(h1 + 1) * HD]], axis=1)
            cols.append(_to_sbuf_chunks(np.ascontiguousarray(Wp, f), NCH))
            bp[p, 0:64, 0] = bvec[h0 * HD:(h0 + 1) * HD]
            bp[p, 64:128, 0] = bvec[h1 * HD:(h1 + 1) * HD]
        return np.concatenate(cols, axis=1), bp
    wq, bqp = pack(Wq, bq)
    wk, bkp = pack(Wk, bk)
    wv = _to_sbuf_chunks(np.ascontiguousarray(Wv[:, g * CLOC:(g + 1) * CLOC], f), NCH)
    wo = _to_sbuf_chunks(np.ascontiguousarray(Wo[g * CLOC:(g + 1) * CLOC, :], f), 2)
    return {"xt": xt, "wq": wq, "wk": wk, "wv": wv, "wo": wo,
            "bq": bqp, "bk": bkp,
            "ones": np.ones((128, NTT, NHL, 8), ml_dtypes.bfloat16),
            "sels": _sels(), "selc": _selc(),
            "tri": np.triu(np.ones((128, 128))).astype(ml_dtypes.bfloat16)}


def _run(inputs, trace=False, tmpdir=None):
    if "nc" not in _cache:
        _cache["nc"] = _build()
    nc = _cache["nc"]
    args = [np.asarray(inputs[k], np.float32) for k in
            ("x", "Wq", "bq", "Wk", "bk", "Wv", "bv", "Wo", "bo")]
    x, Wq, bq, Wk, bk, Wv, bv, Wo, bo = args
    in_maps = [_prep_core_inputs(c // 4, c % 4, x, Wq, bq, Wk, bk, Wv, bv, Wo, bo)
               for c in range(8)]
    res = bass_utils.run_bass_kernel_spmd(nc, in_maps, core_ids=list(range(8)),
                                          trace=trace, tmpdir=tmpdir)
    corr = (bv.astype(np.float64) @ Wo.astype(np.float64) + bo).astype(np.float32)
    out = np.empty((B, T, C), np.float32)
    for b in range(B):
        acc = np.zeros((T, C), np.float64)
        for g in range(4):
            acc += res.results[b * 4 + g]["out"]
        out[b] = (acc + corr).astype(np.float32)
    return out, res


def kernel(x, Wq, bq, Wk, bk, Wv, bv, Wo, bo):
    out, _ = _run(dict(x=x, Wq=Wq, bq=bq, Wk=Wk, bk=bk, Wv=Wv, bv=bv,
                       Wo=Wo, bo=bo))
    return out


def run_profiled(x, Wq, bq, Wk, bk, Wv, bv, Wo, bo, tmpdir=None):
    out, res = _run(dict(x=x, Wq=Wq, bq=bq, Wk=Wk, bk=bk, Wv=Wv, bv=bv,
                         Wo=Wo, bo=bo), trace=True, tmpdir=tmpdir)
    return out, res.exec_time_ns, res



# revision 16
# speedup vs baseline: 1.5107x; 1.5107x over previous
"""Causal self-attention kernel for Trainium2, 8-way sharded (v2).

Problem: B=2, T=2048, C=1024, NH=16, hd=64. fp32 in/out.

Sharding: core = (batch b, head-group g of 4 heads). Each core computes its
4 heads' attention for its batch plus the partial output projection
y_local @ Wo[g*256:(g+1)*256, :]; the host sums the 4 partials per batch
(biases bv/bo are folded in exactly via a host-side correction row).

v2 design (vs v1 at ~302us):
  - All matmul operands bf16 (x, weights, q/k, P, V, y): halves input DMA,
    enables FWL weight loads; PSUM accumulation stays fp32.
  - x streamed window-major so the first q/k projection only waits for
    ~2.5MB of DMA instead of the full input set.
  - Score matmuls (K=64) emitted h-interleaved so the two heads of a pair
    run concurrently in different PE row groups (tile_position derives
    from base partitions 0/64): ~2x on the score stream.
  - Causal trimming: diagonal-band chunks compute only live tq columns in
    the score and PV matmuls. exp still covers the full tile; the dead
    columns hold garbage that PV never reads (no masking memsets needed).
  - Denominators: V_aug ones column at 64+h per head -> acc row 64+h;
    per-window: 2 row copies + reciprocal_approx_fast + one f32r
    broadcast matmul + 2 muls (v1 paid 16x 924ns fp32 selector matmuls
    and 3.3us batched reciprocals).
  - Output projection emitted per-window as PE filler inside the pair-1
    attention phase; bf16 out DMA overlaps compute; host sums partials.
  - Fillers (v tiles, pair-1 projections, out tiles) paced by an
    ACT-deficit model plus a per-slot minimum drain so no bulk stalls.
"""
import contextlib

import ml_dtypes
import numpy as np

import concourse.bass as bass
import concourse.tile as tile
from concourse import bacc, mybir
from concourse import bass_utils

bass_utils.upload_artifacts = lambda tmpdir: "local://skipped"

B, T, C = 2, 2048, 1024
NH, HD = 16, 64
NHL = 4            # heads per core
CLOC = NHL * HD    # 256 local channels
NCH = C // 128     # 8 contraction chunks
TQW = 512          # tq window
NW = T // TQW      # 4 windows
NTT = T // 128     # 16 t-tiles / tk-chunks
VSTR = HD + 1      # 65: v cols + ones col (denominator lands on acc row 64)
LAG = 2            # PV trails S^T by this many groups
XW = NCH * TQW     # x cols per window
F32R = mybir.dt.float32r
F32 = mybir.dt.float32
BF16 = mybir.dt.bfloat16
EXP = mybir.ActivationFunctionType.Exp

_cache = {}


def _trim(c, w):
    """Dead tq columns at the left edge of window w for tk-chunk c."""
    return 128 * (c - 4 * w) if c >= 4 * w else 0


def _build():
    nc = bacc.Bacc("TRN2", target_bir_lowering=False, debug=False, num_devices=8)

    xt_ap = nc.dram_tensor("xt", [128, NW * XW], BF16, kind="ExternalInput").ap()
    wq_ap = nc.dram_tensor("wq", [128, 2 * NCH * 128], BF16, kind="ExternalInput").ap()
    wk_ap = nc.dram_tensor("wk", [128, 2 * NCH * 128], BF16, kind="ExternalInput").ap()
    wv_ap = nc.dram_tensor("wv", [128, NCH * CLOC], BF16, kind="ExternalInput").ap()
    wo_ap = nc.dram_tensor("wo", [128, 2 * C], BF16, kind="ExternalInput").ap()
    bq_ap = nc.dram_tensor("bq", [2, 128, 1], F32, kind="ExternalInput").ap()
    bk_ap = nc.dram_tensor("bk", [2, 128, 1], F32, kind="ExternalInput").ap()
    ones_ap = nc.dram_tensor("ones", [128, NTT, NHL, 1], BF16, kind="ExternalInput").ap()
    tri_ap = nc.dram_tensor("tri", [128, 128], BF16, kind="ExternalInput").ap()
    bc_ap = nc.dram_tensor("bcast", [128, 128], F32R, kind="ExternalInput").ap()
    out_ap = nc.dram_tensor("out", [T, C], BF16, kind="ExternalOutput").ap()

    with tile.TileContext(nc) as tc, contextlib.ExitStack() as ctx:
        sb = ctx.enter_context(tc.tile_pool(name="sb", bufs=1))
        rs_pool = ctx.enter_context(tc.tile_pool(name="rsp", bufs=2))
        ost_pool = ctx.enter_context(tc.tile_pool(name="ost", bufs=3))
        pt_pool = ctx.enter_context(tc.tile_pool(name="ptp", bufs=8))
        ps = ctx.enter_context(tc.tile_pool(name="ps", bufs=1, space="PSUM"))

        # ---- persistent SBUF tensors ----
        wqs = sb.tile([128, 2 * NCH * 128], BF16, tag="wqs")
        wks = sb.tile([128, 2 * NCH * 128], BF16, tag="wks")
        wvs = sb.tile([128, NCH * CLOC], BF16, tag="wvs")
        wos = sb.tile([128, 2 * C], BF16, tag="wos")
        xts = sb.tile([128, NW * XW], BF16, tag="xts")
        qts = [sb.tile([128, T], BF16, tag=f"qt{p}", name=f"qt{p}") for p in range(2)]
        kts = [sb.tile([128, T], BF16, tag=f"kt{p}", name=f"kt{p}") for p in range(2)]
        vna = sb.tile([128, NTT * NHL * VSTR], BF16, tag="vna")
        yts = [sb.tile([128, T], BF16, tag=f"yt{p}", name=f"yt{p}") for p in range(2)]
        bqs = [sb.tile([128, 1], F32, tag=f"bq{p}", name=f"bqs{p}") for p in range(2)]
        bks = [sb.tile([128, 1], F32, tag=f"bk{p}", name=f"bks{p}") for p in range(2)]
        tri = sb.tile([128, 128], BF16, tag="tri")
        bcs = sb.tile([128, 128], F32R, tag="bcs")

        # ---- input DMAs: first matmuls need wq + x window 0 ----
        nc.sync.dma_start(wqs[:], wq_ap[:])
        nc.gpsimd.dma_start(xts[:, 0:XW], xt_ap[:, 0:XW])
        nc.scalar.dma_start(xts[:, XW:2 * XW], xt_ap[:, XW:2 * XW])
        nc.sync.dma_start(wks[:], wk_ap[:])
        nc.gpsimd.dma_start(wvs[:], wv_ap[:])
        nc.scalar.dma_start(xts[:, 2 * XW:3 * XW], xt_ap[:, 2 * XW:3 * XW])
        nc.gpsimd.dma_start(xts[:, 3 * XW:4 * XW], xt_ap[:, 3 * XW:4 * XW])
        for p in range(2):
            nc.sync.dma_start(bqs[p][:], bq_ap[p])
            nc.sync.dma_start(bks[p][:], bk_ap[p])
        nc.sync.dma_start(tri[:], tri_ap[:])
        nc.sync.dma_start(bcs[:], bc_ap[:])
        vna4 = vna[:].rearrange("p (t h v) -> p t h v", t=NTT, h=NHL)
        nc.sync.dma_start(vna4[:, :, :, HD:HD + 1], ones_ap[:])
        nc.scalar.dma_start(wos[:], wo_ap[:])

        pt_tiles = {}

        def xsl(w, c, off=0, ln=TQW):
            base = (w * NCH + c) * TQW
            return xts[:, base + off: base + off + ln]

        # ---------- emission primitives ----------
        def warm():
            wtile = sb.tile([128, 640], BF16, tag="warm")
            wjunk = sb.tile([128, 8], F32, tag="wjunk")
            nc.vector.memset(wtile[:], 0.0)
            wp = ps.tile([128, TQW], F32, tag="work", bufs=2, name="warm_ps")
            for i in range(16):
                nc.tensor.matmul(wp[:], wtile[:, 0:128], wtile[:, 128:640],
                                 start=True, stop=True)
            # preload the ACT exp table while DMA streams in
            nc.scalar.activation(wjunk[:], wp[:, 0:8], EXP, scale=0.125)

        def qk_window(p, ty, w):
            wsb, dst, bias = ((wqs, qts[p], bqs[p]), (wks, kts[p], bks[p]))[ty]
            tag = "st" if p == 0 else "work"
            acc = ps.tile([128, TQW], F32, tag=tag, bufs=2, name=f"qk{p}{ty}{w}")
            for c in range(NCH):
                nc.tensor.matmul(
                    acc[:], wsb[:, (p * NCH + c) * 128:(p * NCH + c + 1) * 128],
                    xsl(w, c),
                    start=(c == 0), stop=(c == NCH - 1))
            nc.vector.tensor_scalar_add(dst[:, w * TQW:(w + 1) * TQW],
                                        acc[:], bias[:])

        def v_tile(tt):
            w0, r = divmod(tt, 4)
            acc = ps.tile([128, CLOC], F32, tag="work", bufs=2, name=f"v{tt}")
            for c in range(NCH):
                nc.tensor.matmul(acc[:], xsl(w0, c, r * 128, 128),
                                 wvs[:, c * CLOC:(c + 1) * CLOC],
                                 start=(c == 0), stop=(c == NCH - 1))
            base = tt * NHL * VSTR
            dst = vna[:, base:base + NHL * VSTR].rearrange("p (h d) -> p h d", h=NHL)
            nc.vector.tensor_copy(dst[:, :, 0:HD],
                                  acc[:].rearrange("p (h d) -> p h d", h=NHL))

        def st_pair(p, w, g):
            # 4 score MMs, h-interleaved so rows 0-63 / 64-127 overlap
            qt, kt = qts[p], kts[p]
            sts = [ps.tile([128, 1024], F32, tag="st", bufs=2,
                           name=f"st{p}{w}{g}{h}") for h in range(2)]
            for j in range(2):
                c = 2 * g + j
                tr = _trim(c, w)
                for h in range(2):
                    nc.tensor.matmul(
                        sts[h][:, j * TQW + tr:(j + 1) * TQW],
                        kt[h * 64:(h + 1) * 64, c * 128:(c + 1) * 128],
                        qt[h * 64:(h + 1) * 64, w * TQW + tr:(w + 1) * TQW],
                        start=True, stop=True)
            for h in range(2):
                pt = pt_pool.tile([128, 1024], BF16, tag="pt", name=f"pt{p}{w}{g}{h}")
                nc.scalar.activation(pt[:], sts[h][:], EXP, scale=0.125)
                for j in range(2):
                    c = 2 * g + j
                    if c >= 4 * w:  # diagonal chunk: mask its triangle block
                        tr = _trim(c, w)
                        dslc = pt[:, j * TQW + tr: j * TQW + tr + 128]
                        nc.gpsimd.tensor_mul(dslc, dslc, tri[:])
                pt_tiles[(p, w, g, h)] = pt

        def pv_group(p, w, g, accs):
            nchunks = 4 * (w + 1)
            for h in range(2):
                pt = pt_tiles.pop((p, w, g, h))
                for j in range(2):
                    c = 2 * g + j
                    tr = _trim(c, w)
                    vbase = (c * NHL + 2 * p + h) * VSTR
                    nc.tensor.matmul(
                        accs[h][0:VSTR, tr:TQW],
                        vna[:, vbase:vbase + VSTR],
                        pt[:, j * TQW + tr:(j + 1) * TQW],
                        start=(c == 0), stop=(c == nchunks - 1),
                        skip_group_check=(tr > 0))

        def pv_tail(p, w, h, accs, rstage):
            nc.vector.tensor_copy(
                yts[p][h * 64:(h + 1) * 64, w * TQW:(w + 1) * TQW],
                accs[h][0:HD, :])
            nc.vector.tensor_copy(rstage[HD:HD + 1, h * TQW:(h + 1) * TQW],
                                  accs[h][HD:HD + 1, :])

        def norm_a(p, w, rstage, rsb):
            # broadcast each head's denominator row to all 128 partitions,
            # then reciprocal on the full tile (recip_approx_fast needs a
            # full-partition fp32 operand to behave)
            for h in range(2):
                RD = ps.tile([128, TQW], F32, tag="work", bufs=2,
                             name=f"RD{p}{w}{h}")
                nc.tensor.matmul(RD[:], bcs[HD:HD + 1, 0:128],
                                 rstage[HD:HD + 1, h * TQW:(h + 1) * TQW],
                                 start=True, stop=True)
                nc.vector.reciprocal_approx_fast(
                    rsb[:, h * TQW:(h + 1) * TQW], RD[:])

        def norm_b(p, w, rsb):
            for h in range(2):
                yslc = yts[p][h * 64:(h + 1) * 64, w * TQW:(w + 1) * TQW]
                nc.vector.tensor_mul(
                    yslc, yslc,
                    rsb[h * 64:(h + 1) * 64, h * TQW:(h + 1) * TQW])

        def out_tile(tt):
            for nh in range(2):
                po = ps.tile([128, TQW], F32, tag="work", bufs=2,
                             name=f"po{tt}{nh}")
                for cc in range(2):
                    nc.tensor.matmul(
                        po[:], yts[cc][:, tt * 128:(tt + 1) * 128],
                        wos[:, cc * C + nh * TQW: cc * C + nh * TQW + TQW],
                        start=(cc == 0), stop=(cc == 1))
                ost = ost_pool.tile([128, TQW], BF16, tag="ost", name=f"o{tt}{nh}")
                if (2 * tt + nh) % 2 == 0:
                    nc.vector.tensor_copy(ost[:], po[:])
                else:
                    nc.scalar.copy(ost[:], po[:])
                deng = (nc.sync, nc.gpsimd, nc.scalar)[(2 * tt + nh) % 3]
                deng.dma_start(
                    out_ap[tt * 128:(tt + 1) * 128, nh * TQW:(nh + 1) * TQW],
                    ost[:])

        # ---------- fused schedule ----------
        filler = []
        for tt in range(NTT):
            filler.append(("v", tt, lambda tt=tt: v_tile(tt), 0.90))
        for w in range(NW):
            for ty in range(2):
                filler.append(("qk1", (ty, w),
                               lambda ty=ty, w=w: qk_window(1, ty, w), 1.75))

        state = {"deficit": 0.0}

        def emit(item):
            kind, ident, fn, cost = item
            fn()
            state["deficit"] -= cost

        def pull_matching(pred):
            i = 0
            while i < len(filler):
                if pred(filler[i][0], filler[i][1]):
                    emit(filler.pop(i))
                else:
                    i += 1

        def pace(min_pull):
            pulled = 0
            while filler and (state["deficit"] > 0.8 or pulled < min_pull):
                emit(filler.pop(0))
                pulled += 1

        warm()
        for w in range(NW):
            qk_window(0, 0, w)
            qk_window(0, 1, w)

        norm_pending = []
        for p in range(2):
            for w in range(NW):
                if p == 1:
                    # pair-1 scores for window w need its q/k projections
                    pull_matching(lambda k, i: k == "qk1" and i[1] == w)
                ngroups = 2 * (w + 1)
                accs = [ps.tile([128, TQW], F32, tag=f"acc{h}", bufs=1,
                                name=f"acc{p}{w}{h}") for h in range(2)]
                rstage = rs_pool.tile([128, 2 * TQW], F32R, tag="rst",
                                      name=f"rs{p}{w}")
                rsb = rs_pool.tile([128, 2 * TQW], F32, tag="rsb",
                                   name=f"rb{p}{w}")
                for g in range(ngroups + LAG):
                    if g < ngroups:
                        # PV of group g needs v tiles for chunks 2g, 2g+1
                        pull_matching(lambda k, i, g=g: k == "v" and i <= 2 * g + 1)
                        st_pair(p, w, g)
                        pe = (1024 - _trim(2 * g, w) - _trim(2 * g + 1, w)) / 2400
                        state["deficit"] += 2.2 - pe
                    if g == 1 and norm_pending:
                        norm_pending.pop(0)()
                        state["deficit"] -= 0.35
                    if g >= LAG:
                        gg = g - LAG
                        pv_group(p, w, gg, accs)
                        pe = (1024 - _trim(2 * gg, w) - _trim(2 * gg + 1, w)) / 2400
                        state["deficit"] -= pe
                    pace(min_pull=1)
                for h in range(2):
                    pv_tail(p, w, h, accs, rstage)
                norm_a(p, w, rstage, rsb)

                def norm_release(p=p, w=w, rsb=rsb):
                    # out tiles become pullable only after the normalize
                    # muls are emitted (else they'd read unnormalized y)
                    norm_b(p, w, rsb)
                    if p == 1:
                        for tt in range(4 * w, 4 * w + 4):
                            filler.append(("out", tt,
                                           lambda tt=tt: out_tile(tt), 0.86))

                if p == 1 and w == NW - 1:
                    norm_release()
                else:
                    norm_pending.append(norm_release)
        while norm_pending:
            norm_pending.pop(0)()
        pull_matching(lambda k, i: True)

    nc.compile()
    return nc


def _to_sbuf_chunks(a, nch):
    """[nch*128, F] row-major -> [128, nch*F] SBUF-native layout."""
    n, fdim = a.shape
    assert n == nch * 128
    return np.ascontiguousarray(
        a.reshape(nch, 128, fdim).transpose(1, 0, 2).reshape(128, nch * fdim))


def _pack_x(xb):
    """x[b] [T, C] fp32 -> window-major x^T [128, NW*NCH*TQW] bf16."""
    xT = np.ascontiguousarray(xb.T).astype(ml_dtypes.bfloat16)      # [C, T]
    x4 = xT.reshape(NCH, 128, NW, TQW).transpose(1, 2, 0, 3)        # [128,w,c,t]
    return np.ascontiguousarray(x4.reshape(128, NW * XW))


def _prep_core_inputs(g, xt, Wq, bq, Wk, bk, Wv, bv, Wo, bo):
    bf = ml_dtypes.bfloat16
    f = np.float32

    def pack_qk(W, bvec):
        cols = []
        bp = np.empty((2, 128, 1), f)
        for p in range(2):
            h0, h1 = 4 * g + 2 * p, 4 * g + 2 * p + 1
            Wp = np.concatenate([W[:, h0 * HD:(h0 + 1) * HD],
                                 W[:, h1 * HD:(h1 + 1) * HD]], axis=1)
            cols.append(_to_sbuf_chunks(np.ascontiguousarray(Wp, f), NCH))
            bp[p, 0:64, 0] = bvec[h0 * HD:(h0 + 1) * HD]
            bp[p, 64:128, 0] = bvec[h1 * HD:(h1 + 1) * HD]
        return np.concatenate(cols, axis=1).astype(bf), bp

    wq, bqp = pack_qk(Wq, bq)
    wk, bkp = pack_qk(Wk, bk)
    wv = _to_sbuf_chunks(
        np.ascontiguousarray(Wv[:, g * CLOC:(g + 1) * CLOC], f), NCH).astype(bf)
    wo = _to_sbuf_chunks(
        np.ascontiguousarray(Wo[g * CLOC:(g + 1) * CLOC, :], f), 2).astype(bf)
    ones = np.ones((128, NTT, NHL, 1), bf)
    bc = np.zeros((128, 128), f)
    bc[64, :] = 1.0
    return {"xt": xt, "wq": wq, "wk": wk, "wv": wv, "wo": wo,
            "bq": bqp, "bk": bkp, "ones": ones,
            "tri": np.triu(np.ones((128, 128))).astype(bf), "bcast": bc}


def _run(inputs, trace=False, tmpdir=None):
    if "nc" not in _cache:
        _cache["nc"] = _build()
    nc = _cache["nc"]
    args = [np.asarray(inputs[k], np.float32) for k in
            ("x", "Wq", "bq", "Wk", "bk", "Wv", "bv", "Wo", "bo")]
    x, Wq, bq, Wk, bk, Wv, bv, Wo, bo = args
    xt_b = [_pack_x(x[b]) for b in range(B)]
    in_maps = [_prep_core_inputs(c % 4, xt_b[c // 4],
                                 Wq, bq, Wk, bk, Wv, bv, Wo, bo)
               for c in range(8)]
    res = bass_utils.run_bass_kernel_spmd(nc, in_maps, core_ids=list(range(8)),
                                          trace=trace, tmpdir=tmpdir)
    corr = (bv.astype(np.float64) @ Wo.astype(np.float64) + bo).astype(np.float32)
    out = np.empty((B, T, C), np.float32)
    for b in range(B):
        acc = np.zeros((T, C), np.float32)
        for g in range(4):
            acc += np.asarray(res.results[b * 4 + g]["out"], np.float32)
        out[b] = acc + corr
    return out, res


def kernel(x, Wq, bq, Wk, bk, Wv, bv, Wo, bo):
    out, _ = _run(dict(x=x, Wq=Wq, bq=bq, Wk=Wk, bk=bk, Wv=Wv, bv=bv,
                       Wo=Wo, bo=bo))
    return out


def run_profiled(x, Wq, bq, Wk, bk, Wv, bv, Wo, bo, tmpdir=None):
    out, res = _run(dict(x=x, Wq=Wq, bq=bq, Wk=Wk, bk=bk, Wv=Wv, bv=bv,
                         Wo=Wo, bo=bo), trace=True, tmpdir=tmpdir)
    return out, res.exec_time_ns, res


# revision 28
# speedup vs baseline: 1.5231x; 1.0082x over previous
"""Causal self-attention kernel for Trainium2, 8-way sharded (v2).

Problem: B=2, T=2048, C=1024, NH=16, hd=64. fp32 in/out.

Sharding: core = (batch b, head-group g of 4 heads). Each core computes its
4 heads' attention for its batch plus the partial output projection
y_local @ Wo[g*256:(g+1)*256, :]; the host sums the 4 partials per batch
(biases bv/bo are folded in exactly via a host-side correction row).

v2 design (vs v1 at ~302us):
  - All matmul operands bf16 (x, weights, q/k, P, V, y): halves input DMA,
    enables FWL weight loads; PSUM accumulation stays fp32.
  - x streamed window-major so the first q/k projection only waits for
    ~2.5MB of DMA instead of the full input set.
  - Score matmuls (K=64) emitted h-interleaved so the two heads of a pair
    run concurrently in different PE row groups (tile_position derives
    from base partitions 0/64): ~2x on the score stream.
  - Causal trimming: diagonal-band chunks compute only live tq columns in
    the score and PV matmuls. exp still covers the full tile; the dead
    columns hold garbage that PV never reads (no masking memsets needed).
  - Denominators: V_aug ones column at 64+h per head -> acc row 64+h;
    per-window: 2 row copies + reciprocal_approx_fast + one f32r
    broadcast matmul + 2 muls (v1 paid 16x 924ns fp32 selector matmuls
    and 3.3us batched reciprocals).
  - Output projection emitted per-window as PE filler inside the pair-1
    attention phase; bf16 out DMA overlaps compute; host sums partials.
  - Fillers (v tiles, pair-1 projections, out tiles) paced by an
    ACT-deficit model plus a per-slot minimum drain so no bulk stalls.
"""
import contextlib

import ml_dtypes
import numpy as np

import concourse.bass as bass
import concourse.tile as tile
from concourse import bacc, mybir
from concourse import bass_utils

bass_utils.upload_artifacts = lambda tmpdir: "local://skipped"

B, T, C = 2, 2048, 1024
NH, HD = 16, 64
NHL = 4            # heads per core
CLOC = NHL * HD    # 256 local channels
NCH = C // 128     # 8 contraction chunks
TQW = 512          # tq window
NW = T // TQW      # 4 windows
NTT = T // 128     # 16 t-tiles / tk-chunks
VSTR = 128         # v cols + ones col at 64 (denominator), zero-pad to 128 for FWL
LAG = 2            # PV trails S^T by this many groups
XW = NCH * TQW     # x cols per window
F32R = mybir.dt.float32r
F32 = mybir.dt.float32
BF16 = mybir.dt.bfloat16
EXP = mybir.ActivationFunctionType.Exp

_cache = {}


def _trim(c, w):
    """Dead tq columns at the left edge of window w for tk-chunk c."""
    return 128 * (c - 4 * w) if c >= 4 * w else 0


def _build():
    nc = bacc.Bacc("TRN2", target_bir_lowering=False, debug=False, num_devices=8)

    xt_ap = nc.dram_tensor("xt", [128, NW * XW], BF16, kind="ExternalInput").ap()
    wq_ap = nc.dram_tensor("wq", [128, 2 * NCH * 128], BF16, kind="ExternalInput").ap()
    wk_ap = nc.dram_tensor("wk", [128, 2 * NCH * 128], BF16, kind="ExternalInput").ap()
    wv_ap = nc.dram_tensor("wv", [128, NCH * CLOC], BF16, kind="ExternalInput").ap()
    wo_ap = nc.dram_tensor("wo", [128, 2 * C], BF16, kind="ExternalInput").ap()
    bq_ap = nc.dram_tensor("bq", [2, 128, 1], F32, kind="ExternalInput").ap()
    bk_ap = nc.dram_tensor("bk", [2, 128, 1], F32, kind="ExternalInput").ap()
    ones_ap = nc.dram_tensor("ones", [128, NTT, NHL, VSTR - HD], BF16,
                             kind="ExternalInput").ap()
    tri_ap = nc.dram_tensor("tri", [128, 128], BF16, kind="ExternalInput").ap()
    bc_ap = nc.dram_tensor("bcast", [128, 128], F32R, kind="ExternalInput").ap()
    out_ap = nc.dram_tensor("out", [T, C], BF16, kind="ExternalOutput").ap()

    with tile.TileContext(nc) as tc, contextlib.ExitStack() as ctx:
        sb = ctx.enter_context(tc.tile_pool(name="sb", bufs=1))
        rs_pool = ctx.enter_context(tc.tile_pool(name="rsp", bufs=2))
        ost_pool = ctx.enter_context(tc.tile_pool(name="ost", bufs=3))
        pt_pool = ctx.enter_context(tc.tile_pool(name="ptp", bufs=8))
        ps = ctx.enter_context(tc.tile_pool(name="ps", bufs=1, space="PSUM"))

        # ---- persistent SBUF tensors ----
        wqs = sb.tile([128, 2 * NCH * 128], BF16, tag="wqs")
        wks = sb.tile([128, 2 * NCH * 128], BF16, tag="wks")
        wvs = sb.tile([128, NCH * CLOC], BF16, tag="wvs")
        wos = sb.tile([128, 2 * C], BF16, tag="wos")
        xts = sb.tile([128, NW * XW], BF16, tag="xts")
        qts = [sb.tile([128, T], BF16, tag=f"qt{p}", name=f"qt{p}") for p in range(2)]
        kts = [sb.tile([128, T], BF16, tag=f"kt{p}", name=f"kt{p}") for p in range(2)]
        vna = sb.tile([128, NTT * NHL * VSTR], BF16, tag="vna")
        yts = [sb.tile([128, T], BF16, tag=f"yt{p}", name=f"yt{p}") for p in range(2)]
        bqs = [sb.tile([128, 1], F32, tag=f"bq{p}", name=f"bqs{p}") for p in range(2)]
        bks = [sb.tile([128, 1], F32, tag=f"bk{p}", name=f"bks{p}") for p in range(2)]
        tri = sb.tile([128, 128], BF16, tag="tri")
        bcs = sb.tile([128, 128], F32R, tag="bcs")

        # ---- input DMAs: first matmuls need wq + x window 0 ----
        nc.sync.dma_start(wqs[:], wq_ap[:])
        nc.gpsimd.dma_start(xts[:, 0:XW], xt_ap[:, 0:XW])
        nc.scalar.dma_start(xts[:, XW:2 * XW], xt_ap[:, XW:2 * XW])
        nc.sync.dma_start(wks[:], wk_ap[:])
        nc.gpsimd.dma_start(wvs[:], wv_ap[:])
        nc.scalar.dma_start(xts[:, 2 * XW:3 * XW], xt_ap[:, 2 * XW:3 * XW])
        nc.gpsimd.dma_start(xts[:, 3 * XW:4 * XW], xt_ap[:, 3 * XW:4 * XW])
        for p in range(2):
            nc.sync.dma_start(bqs[p][:], bq_ap[p])
            nc.sync.dma_start(bks[p][:], bk_ap[p])
        nc.sync.dma_start(tri[:], tri_ap[:])
        nc.sync.dma_start(bcs[:], bc_ap[:])
        vna4 = vna[:].rearrange("p (t h v) -> p t h v", t=NTT, h=NHL)
        nc.sync.dma_start(vna4[:, :, :, HD:VSTR], ones_ap[:])
        nc.scalar.dma_start(wos[:], wo_ap[:])

        pt_tiles = {}

        def xsl(w, c, off=0, ln=TQW):
            base = (w * NCH + c) * TQW
            return xts[:, base + off: base + off + ln]

        # ---------- emission primitives ----------
        def warm():
            wtile = sb.tile([128, 640], BF16, tag="warm")
            wjunk = sb.tile([128, 8], F32, tag="wjunk")
            nc.vector.memset(wtile[:], 0.0)
            wp = ps.tile([128, TQW], F32, tag="work", bufs=2, name="warm_ps")
            for i in range(16):
                nc.tensor.matmul(wp[:, 0:256], wtile[:, 0:128],
                                 wtile[:, 128:384], start=True, stop=True)
            # preload the ACT exp table while DMA streams in
            nc.scalar.activation(wjunk[:], wp[:, 0:8], EXP, scale=0.125)

        def qk_window(p, ty, w):
            wsb, dst, bias = ((wqs, qts[p], bqs[p]), (wks, kts[p], bks[p]))[ty]
            tag = "st" if (p == 0 and w == 0) else "work"
            acc = ps.tile([128, TQW], F32, tag=tag, bufs=2, name=f"qk{p}{ty}{w}")
            for c in range(NCH):
                nc.tensor.matmul(
                    acc[:], wsb[:, (p * NCH + c) * 128:(p * NCH + c + 1) * 128],
                    xsl(w, c),
                    start=(c == 0), stop=(c == NCH - 1))
            nc.vector.tensor_scalar_add(dst[:, w * TQW:(w + 1) * TQW],
                                        acc[:], bias[:])

        def v_tile(tt):
            w0, r = divmod(tt, 4)
            acc = ps.tile([128, CLOC], F32, tag="work", bufs=2, name=f"v{tt}")
            for c in range(NCH):
                nc.tensor.matmul(acc[:], xsl(w0, c, r * 128, 128),
                                 wvs[:, c * CLOC:(c + 1) * CLOC],
                                 start=(c == 0), stop=(c == NCH - 1))
            base = tt * NHL * VSTR
            dst = vna[:, base:base + NHL * VSTR].rearrange("p (h d) -> p h d", h=NHL)
            nc.vector.tensor_copy(dst[:, :, 0:HD],
                                  acc[:].rearrange("p (h d) -> p h d", h=NHL))

        def st_pair(p, w, g):
            # 4 score MMs, h-interleaved so rows 0-63 / 64-127 overlap
            qt, kt = qts[p], kts[p]
            sts = [ps.tile([128, 1024], F32, tag="st", bufs=2,
                           name=f"st{p}{w}{g}{h}") for h in range(2)]
            for j in range(2):
                c = 2 * g + j
                tr = _trim(c, w)
                for h in range(2):
                    nc.tensor.matmul(
                        sts[h][:, j * TQW + tr:(j + 1) * TQW],
                        kt[h * 64:(h + 1) * 64, c * 128:(c + 1) * 128],
                        qt[h * 64:(h + 1) * 64, w * TQW + tr:(w + 1) * TQW],
                        start=True, stop=True)
            split = _trim(2 * g, w) > 0  # both chunks trimmed: exp per live slice
            for h in range(2):
                pt = pt_pool.tile([128, 1024], BF16, tag="pt", name=f"pt{p}{w}{g}{h}")
                if split:
                    for j in range(2):
                        tr = _trim(2 * g + j, w)
                        sl = slice(j * TQW + tr, (j + 1) * TQW)
                        nc.scalar.activation(pt[:, sl], sts[h][:, sl], EXP,
                                             scale=0.125)
                else:
                    nc.scalar.activation(pt[:], sts[h][:], EXP, scale=0.125)
                for j in range(2):
                    c = 2 * g + j
                    if c >= 4 * w:  # diagonal chunk: mask its triangle block
                        tr = _trim(c, w)
                        dslc = pt[:, j * TQW + tr: j * TQW + tr + 128]
                        nc.gpsimd.tensor_mul(dslc, dslc, tri[:])
                pt_tiles[(p, w, g, h)] = pt

        def pv_group(p, w, g, accs):
            nchunks = 4 * (w + 1)
            for h in range(2):
                pt = pt_tiles.pop((p, w, g, h))
                for j in range(2):
                    c = 2 * g + j
                    tr = _trim(c, w)
                    vbase = (c * NHL + 2 * p + h) * VSTR
                    nc.tensor.matmul(
                        accs[h][0:VSTR, tr:TQW],
                        vna[:, vbase:vbase + VSTR],
                        pt[:, j * TQW + tr:(j + 1) * TQW],
                        start=(c == 0), stop=(c == nchunks - 1),
                        skip_group_check=(tr > 0))

        def pv_tail(p, w, h, accs, rstage):
            nc.vector.tensor_copy(
                yts[p][h * 64:(h + 1) * 64, w * TQW:(w + 1) * TQW],
                accs[h][0:HD, :])
            nc.vector.tensor_copy(rstage[HD:HD + 1, h * TQW:(h + 1) * TQW],
                                  accs[h][HD:HD + 1, :])

        def norm_a(p, w, rstage, rsb):
            # broadcast each head's denominator row to all 128 partitions,
            # then reciprocal on the full tile (recip_approx_fast needs a
            # full-partition fp32 operand to behave)
            for h in range(2):
                RD = ps.tile([128, TQW], F32, tag="work", bufs=2,
                             name=f"RD{p}{w}{h}")
                nc.tensor.matmul(RD[:], bcs[HD:HD + 1, 0:128],
                                 rstage[HD:HD + 1, h * TQW:(h + 1) * TQW],
                                 start=True, stop=True)
                nc.vector.reciprocal_approx_fast(
                    rsb[:, h * TQW:(h + 1) * TQW], RD[:])

        def norm_b(p, w, rsb):
            for h in range(2):
                yslc = yts[p][h * 64:(h + 1) * 64, w * TQW:(w + 1) * TQW]
                nc.vector.tensor_mul(
                    yslc, yslc,
                    rsb[h * 64:(h + 1) * 64, h * TQW:(h + 1) * TQW])

        def out_tile(tt):
            # cc-outer so both nh matmuls share one weight load
            pos = [ps.tile([128, TQW], F32, tag="work", bufs=2,
                           name=f"po{tt}{nh}") for nh in range(2)]
            for cc in range(2):
                for nh in range(2):
                    nc.tensor.matmul(
                        pos[nh][:], yts[cc][:, tt * 128:(tt + 1) * 128],
                        wos[:, cc * C + nh * TQW: cc * C + nh * TQW + TQW],
                        start=(cc == 0), stop=(cc == 1))
            for nh in range(2):
                ost = ost_pool.tile([128, TQW], BF16, tag="ost", name=f"o{tt}{nh}")
                if (2 * tt + nh) % 2 == 0:
                    nc.vector.tensor_copy(ost[:], pos[nh][:])
                else:
                    nc.scalar.copy(ost[:], pos[nh][:])
                deng = (nc.sync, nc.gpsimd, nc.scalar)[(2 * tt + nh) % 3]
                deng.dma_start(
                    out_ap[tt * 128:(tt + 1) * 128, nh * TQW:(nh + 1) * TQW],
                    ost[:])

        # ---------- fused schedule ----------
        # qk windows beyond (p0,w0) are fillers, force-pulled before the
        # window that needs them; same for v tiles ahead of their PV group.
        filler = []
        for w in range(1, NW):
            for ty in range(2):
                filler.append(("qk", (0, ty, w),
                               lambda ty=ty, w=w: qk_window(0, ty, w), 1.75))
        for tt in range(NTT):
            filler.append(("v", tt, lambda tt=tt: v_tile(tt), 0.90))
        for w in range(NW):
            for ty in range(2):
                filler.append(("qk", (1, ty, w),
                               lambda ty=ty, w=w: qk_window(1, ty, w), 1.75))

        state = {"deficit": 0.0}

        def emit(item):
            kind, ident, fn, cost = item
            fn()
            state["deficit"] -= cost

        def pull_matching(pred):
            i = 0
            while i < len(filler):
                if pred(filler[i][0], filler[i][1]):
                    emit(filler.pop(i))
                else:
                    i += 1

        def pace(min_pull):
            pulled = 0
            while filler and (state["deficit"] > 0.8 or pulled < min_pull):
                emit(filler.pop(0))
                pulled += 1

        warm()
        qk_window(0, 0, 0)
        qk_window(0, 1, 0)

        norm_pending = []
        for p in range(2):
            for w in range(NW):
                # this window's scores need its q/k projections
                pull_matching(
                    lambda k, i, p=p, w=w: k == "qk" and i[0] == p and i[2] == w)
                ngroups = 2 * (w + 1)
                accs = [ps.tile([128, TQW], F32, tag=f"acc{h}", bufs=1,
                                name=f"acc{p}{w}{h}") for h in range(2)]
                rstage = rs_pool.tile([128, 2 * TQW], F32R, tag="rst",
                                      name=f"rs{p}{w}")
                rsb = rs_pool.tile([128, 2 * TQW], F32, tag="rsb",
                                   name=f"rb{p}{w}")
                for g in range(ngroups + LAG):
                    if g < ngroups:
                        # PV of group g needs v tiles for chunks 2g, 2g+1
                        pull_matching(lambda k, i, g=g: k == "v" and i <= 2 * g + 1)
                        st_pair(p, w, g)
                        live = 1024 - _trim(2 * g, w) - _trim(2 * g + 1, w)
                        split = _trim(2 * g, w) > 0
                        act = 2 * (live * 1.05 + (500 if split else 250)) / 1000
                        state["deficit"] += act - live / 2400
                    if g == 1 and norm_pending:
                        norm_pending.pop(0)()
                        state["deficit"] -= 0.35
                    if g >= LAG:
                        gg = g - LAG
                        pv_group(p, w, gg, accs)
                        live = 1024 - _trim(2 * gg, w) - _trim(2 * gg + 1, w)
                        state["deficit"] -= live / 2400
                    pace(min_pull=1)
                for h in range(2):
                    pv_tail(p, w, h, accs, rstage)
                norm_a(p, w, rstage, rsb)

                def norm_release(p=p, w=w, rsb=rsb):
                    # out tiles become pullable only after the normalize
                    # muls are emitted (else they'd read unnormalized y)
                    norm_b(p, w, rsb)
                    if p == 1:
                        for tt in range(4 * w, 4 * w + 4):
                            filler.append(("out", tt,
                                           lambda tt=tt: out_tile(tt), 0.86))

                if p == 1 and w == NW - 1:
                    norm_release()
                else:
                    norm_pending.append(norm_release)
        while norm_pending:
            norm_pending.pop(0)()
        pull_matching(lambda k, i: True)

    nc.compile()
    return nc


def _to_sbuf_chunks(a, nch):
    """[nch*128, F] row-major -> [128, nch*F] SBUF-native layout."""
    n, fdim = a.shape
    assert n == nch * 128
    return np.ascontiguousarray(
        a.reshape(nch, 128, fdim).transpose(1, 0, 2).reshape(128, nch * fdim))


def _pack_x(xb):
    """x[b] [T, C] fp32 -> window-major x^T [128, NW*NCH*TQW] bf16."""
    xT = np.ascontiguousarray(xb.T).astype(ml_dtypes.bfloat16)      # [C, T]
    x4 = xT.reshape(NCH, 128, NW, TQW).transpose(1, 2, 0, 3)        # [128,w,c,t]
    return np.ascontiguousarray(x4.reshape(128, NW * XW))


def _prep_core_inputs(g, xt, Wq, bq, Wk, bk, Wv, bv, Wo, bo):
    bf = ml_dtypes.bfloat16
    f = np.float32

    def pack_qk(W, bvec):
        cols = []
        bp = np.empty((2, 128, 1), f)
        for p in range(2):
            h0, h1 = 4 * g + 2 * p, 4 * g + 2 * p + 1
            Wp = np.concatenate([W[:, h0 * HD:(h0 + 1) * HD],
                                 W[:, h1 * HD:(h1 + 1) * HD]], axis=1)
            cols.append(_to_sbuf_chunks(np.ascontiguousarray(Wp, f), NCH))
            bp[p, 0:64, 0] = bvec[h0 * HD:(h0 + 1) * HD]
            bp[p, 64:128, 0] = bvec[h1 * HD:(h1 + 1) * HD]
        return np.concatenate(cols, axis=1).astype(bf), bp

    wq, bqp = pack_qk(Wq, bq)
    wk, bkp = pack_qk(Wk, bk)
    wv = _to_sbuf_chunks(
        np.ascontiguousarray(Wv[:, g * CLOC:(g + 1) * CLOC], f), NCH).astype(bf)
    wo = _to_sbuf_chunks(
        np.ascontiguousarray(Wo[g * CLOC:(g + 1) * CLOC, :], f), 2).astype(bf)
    ones = np.zeros((128, NTT, NHL, VSTR - HD), bf)
    ones[:, :, :, 0] = 1.0
    bc = np.zeros((128, 128), f)
    bc[64, :] = 1.0
    return {"xt": xt, "wq": wq, "wk": wk, "wv": wv, "wo": wo,
            "bq": bqp, "bk": bkp, "ones": ones,
            "tri": np.triu(np.ones((128, 128))).astype(bf), "bcast": bc}


def _run(inputs, trace=False, tmpdir=None):
    if "nc" not in _cache:
        _cache["nc"] = _build()
    nc = _cache["nc"]
    args = [np.asarray(inputs[k], np.float32) for k in
            ("x", "Wq", "bq", "Wk", "bk", "Wv", "bv", "Wo", "bo")]
    x, Wq, bq, Wk, bk, Wv, bv, Wo, bo = args
    xt_b = [_pack_x(x[b]) for b in range(B)]
    in_maps = [_prep_core_inputs(c % 4, xt_b[c // 4],
                                 Wq, bq, Wk, bk, Wv, bv, Wo, bo)
               for c in range(8)]
    res = bass_utils.run_bass_kernel_spmd(nc, in_maps, core_ids=list(range(8)),
                                          trace=trace, tmpdir=tmpdir)
    corr = (bv.astype(np.float64) @ Wo.astype(np.float64) + bo).astype(np.float32)
    out = np.empty((B, T, C), np.float32)
    for b in range(B):
        acc = np.zeros((T, C), np.float32)
        for g in range(4):
            acc += np.asarray(res.results[b * 4 + g]["out"], np.float32)
        out[b] = acc + corr
    return out, res


def kernel(x, Wq, bq, Wk, bk, Wv, bv, Wo, bo):
    out, _ = _run(dict(x=x, Wq=Wq, bq=bq, Wk=Wk, bk=bk, Wv=Wv, bv=bv,
                       Wo=Wo, bo=bo))
    return out


def run_profiled(x, Wq, bq, Wk, bk, Wv, bv, Wo, bo, tmpdir=None):
    out, res = _run(dict(x=x, Wq=Wq, bq=bq, Wk=Wk, bk=bk, Wv=Wv, bv=bv,
                         Wo=Wo, bo=bo), trace=True, tmpdir=tmpdir)
    return out, res.exec_time_ns, res
